# revision 2
# baseline (speedup 1.0000x reference)
"""Trainium2 Bass kernel for nn_CustomConv2d: 3x3 conv, stride 1, pad 1.

x: [32, 128, 56, 56] f32, kernel: [256, 128, 3, 3] f32, bias: [256] f32
-> out: [32, 256, 56, 56] f32

Strategy: data-parallel over batch (4 images per core on 8 cores).
Per core the conv runs as accumulating matmuls with C_in = 128 as the
PE contraction dim and C_out split into 2 blocks of 128 partitions.
Output pixels are tiled 8 rows (448 cols) per PSUM bank; 7 tiles cover
one image.

v2 loop order is k-outer/tile-inner: for each of the 9 taps, one
weight slice feeds 7 back-to-back matmuls (one per PSUM bank), so the
PE's weight-load stream is amortized 7x instead of paying a reload per
matmul.  Inputs run in bf16 (FWL-eligible weight loads at 2 bf16/read;
~1e-3 rel err vs the 2e-2 gate).  PSUM evictions (bias add) alternate
between the Vector and Scalar engines so bank turnaround never gates
the PE; stores go out on the gpsimd SWDGE queue.
"""

import sys

import numpy as np

try:
    import concourse  # noqa: F401  (provided on PYTHONPATH via axon site)
except ImportError:
    sys.path.insert(0, "/opt/trn_rl_repo")

import concourse.bass as bass
import concourse.mybir as mybir
import concourse.tile as tile
from concourse import bacc
from concourse.bass_utils import run_bass_kernel_spmd

B, C_IN, C_OUT, KS, H, W = 32, 128, 256, 3, 56, 56
N_CORES = 8
B_LOC = B // N_CORES
HP, WP = H + 2, W + 2
ROWS_PER_TILE = 8
N_TILE = ROWS_PER_TILE * W  # 448 <= 512 (one fp32 PSUM bank)
N_TILES = H // ROWS_PER_TILE  # 7
CO_BLOCKS = C_OUT // 128

MODE = "bf16"  # "f32" | "f32r" | "bf16" | "mixw"

_NC_CACHE: dict = {}


def _build_cached(mode: str, repeats: int = 1) -> bass.Bass:
    key = (mode, repeats)
    if key not in _NC_CACHE:
        _NC_CACHE[key] = _build(mode, repeats)
    return _NC_CACHE[key]


def _build(mode: str, repeats: int = 1) -> bass.Bass:
    f32 = mybir.dt.float32
    if mode == "bf16":
        sb_dt = mybir.dt.bfloat16
    elif mode in ("f32r", "mixw"):
        sb_dt = mybir.dt.float32r
    else:
        sb_dt = f32
    w_dt = mybir.dt.bfloat16 if mode == "mixw" else sb_dt

    nc = bacc.Bacc("TRN2", target_bir_lowering=False, debug=False)
    xp_d = nc.dram_tensor("xp", [B_LOC, C_IN, HP, WP], sb_dt, kind="ExternalInput").ap()
    w_d = nc.dram_tensor("w", [C_IN, 9 * C_OUT], w_dt, kind="ExternalInput").ap()
    b_d = nc.dram_tensor("bias", [128, CO_BLOCKS], f32, kind="ExternalInput").ap()
    out_d = nc.dram_tensor("out", [B_LOC, C_OUT, H, W], f32, kind="ExternalOutput").ap()
    out_flat = out_d.rearrange("b c h w -> b c (h w)")

    with tile.TileContext(nc) as tc:
        with (
            tc.tile_pool(name="const", bufs=1) as const,
            tc.tile_pool(name="xpool", bufs=4) as xpool,
            tc.tile_pool(name="opool", bufs=6) as opool,
            tc.tile_pool(name="psum", bufs=8, space="PSUM") as psum,
        ):
            import contextlib

            loop_cm = (
                tc.For_i(0, repeats, 1, hint_engines=(mybir.EngineType.PE,))
                if repeats > 1
                else contextlib.nullcontext()
            )
            with loop_cm:
                # Weights first so the PE's first matmul gates on the
                # smallest possible DMA prefix; then all 4 images; bias is
                # only needed by the first eviction so it loads last.
                wt = const.tile([C_IN, 9 * C_OUT], w_dt, tag="wt", name="wt")
                bt = const.tile([128, CO_BLOCKS], f32)
                nc.sync.dma_start(wt[:], w_d[:])
                xc = []
                for b in range(B_LOC):
                    xt = xpool.tile([C_IN, HP, WP], sb_dt, tag="xt", name="xt")
                    nc.sync.dma_start(
                        xt[:], xp_d[b]
                    )
                    xc.append(xt)
                nc.sync.dma_start(bt[:], b_d[:])

                for b in range(B_LOC):
                    for co in range(CO_BLOCKS):
                        pts = [
                            psum.tile([128, N_TILE], f32, tag="pt", name="pt")
                            for _ in range(N_TILES)
                        ]
                        for k in range(9):
                            kh, kw = divmod(k, KS)
                            wsl = wt[:, (co * 9 + k) * 128 : (co * 9 + k + 1) * 128]
                            for t in range(N_TILES):
                                h0 = ROWS_PER_TILE * t
                                rhs = xc[b][:, h0 + kh : h0 + kh + ROWS_PER_TILE, kw : kw + W]
                                nc.tensor.matmul(
                                    pts[t][:],
                                    wsl,
                                    rhs,
                                    start=(k == 0),
                                    stop=(k == 8),
                                )
                        for t in range(N_TILES):
                            h0 = ROWS_PER_TILE * t
                            ot = opool.tile([128, N_TILE], f32)
                            if t % 2 == 0:
                                nc.vector.tensor_scalar_add(
                                    ot[:], pts[t][:], bt[:, co : co + 1]
                                )
                            else:
                                nc.scalar.activation(
                                    ot[:],
                                    pts[t][:],
                                    mybir.ActivationFunctionType.Copy,
                                    bias=bt[:, co : co + 1],
                                )
                            nc.gpsimd.dma_start(
                                out_flat[b, co * 128 : (co + 1) * 128, h0 * W : h0 * W + N_TILE],
                                ot[:],
                            )
    nc.compile()
    return nc


def _host_prep(x, kernel, bias, mode: str):
    np_dt = np.float32
    w_np_dt = np.float32
    if mode in ("bf16", "mixw"):
        import ml_dtypes

        w_np_dt = ml_dtypes.bfloat16
        if mode == "bf16":
            np_dt = ml_dtypes.bfloat16

    xp = np.zeros((B, C_IN, HP, WP), dtype=np_dt)
    xp[:, :, 1 : 1 + H, 1 : 1 + W] = x
    # w[co, ci, kh, kw] -> w_t[ci, co_blk*9*128 + (kh*3+kw)*128 + co_in]
    w5 = kernel.reshape(CO_BLOCKS, 128, C_IN, KS, KS)
    w_t = np.ascontiguousarray(
        w5.transpose(2, 0, 3, 4, 1).reshape(C_IN, 9 * C_OUT).astype(w_np_dt)
    )
    b_t = np.ascontiguousarray(bias.astype(np.float32).reshape(CO_BLOCKS, 128).T)
    return xp, w_t, b_t


def kernel(x, kernel, bias):  # noqa: A002 - names fixed by harness contract
    x = np.asarray(x, dtype=np.float32)
    kernel = np.asarray(kernel, dtype=np.float32)
    bias = np.asarray(bias, dtype=np.float32)

    nc = _build_cached(MODE)
    xp, w_t, b_t = _host_prep(x, kernel, bias, MODE)
    in_maps = [
        {"xp": xp[c * B_LOC : (c + 1) * B_LOC], "w": w_t, "bias": b_t}
        for c in range(N_CORES)
    ]
    res = run_bass_kernel_spmd(nc, in_maps, core_ids=list(range(N_CORES)))
    out = np.concatenate([r["out"] for r in res.results], axis=0)
    return out
